# revision 3
# baseline (speedup 1.0000x reference)
"""Trainium2 Bass kernel for nn_EnhancedStableNCA.

Strategy (per core, data-parallel over batch: 4 images/core on 8 cores):
- x state lives in SBUF as 8 row-chunk tiles [128, 18, 130] (16 rows + 2 halo
  rows, 128 cols + 2 pad cols); partition p = 32*b + c (b = image-in-core,
  c = channel).  All values bf16.
- The three 3x3 depthwise convs (sobel-x/y, laplacian) are folded into the
  layer-1 1x1-conv matmul: obs = [x,gx,gy,lap] @ W1 becomes
  sum_{dy,dx} A[dy,dx] @ x_shift(dy,dx).  dy-shifts are materialized by DMA
  into a K=48 "y-stack" (3 dy-blocks x 16 ch); dx-shifts are free (AP column
  offsets into the padded stack).  3 matmuls, accumulated in PSUM.
- Two images are row-packed per stack tensor (K=48 at partition bases 0/64)
  so their layer-1 matmuls run concurrently in disjoint PE row groups.
- Layer 3 (M=16) is col-packed: 4 images' matmuls at col positions
  0/32/64/96 into one PSUM bank.
- PSUM evacuation fuses relu+bias (ACT/DVE); mask+bias+0.05 for layer 3
  fuses into one scalar_tensor_tensor from PSUM.
- alive = maxpool3x3(alpha) > 0.01 computed on a compact [32, ...] layout,
  replicated to all channel partitions via DRAM-bounce broadcast DMA.
- update mask m = (rand < 0.5) precomputed on host, broadcast-DMA'd per chunk.
"""

import numpy as np

CH = 16
HID = 128
B, H, W = 32, 128, 128
STEPS = 8
NCORES = 8
NB = B // NCORES          # images per core
NCHUNK = 8                # row chunks per image
RCH = H // NCHUNK         # rows per chunk (16)
PW = W + 2                # padded row width (130)
PR = RCH + 2              # chunk rows incl halo (18)
CC = RCH * W              # compact chunk size (2048)
NSUB = 4                  # 512-px sub-tiles per chunk
SUB = 512

KX = np.array([[-1., 0., 1.], [-2., 0., 2.], [-1., 0., 1.]], np.float32) / 8.0
KY = np.array([[-1., -2., -1.], [0., 0., 0.], [1., 2., 1.]], np.float32) / 8.0
KL = np.array([[0.05, 0.2, 0.05], [0.2, -1.0, 0.2], [0.05, 0.2, 0.05]], np.float32)

_BUILD_CACHE = {}


def _build(steps):
    import os
    import concourse.bacc as bacc
    import concourse.bass as bass
    import concourse.tile as tile
    from concourse import mybir

    F32 = mybir.dt.float32
    BF16 = mybir.dt.bfloat16
    AF = mybir.ActivationFunctionType
    OP = mybir.AluOpType
    _dbg = os.environ.get("NCA_DBG", "")
    loop_n = None
    if isinstance(steps, tuple):          # ("loop", N) timing variant
        loop_n = steps[1]
    elif steps == "loop":
        loop_n = STEPS
    msteps = loop_n if loop_n is not None else STEPS

    nc = bacc.Bacc("TRN2", target_bir_lowering=False, debug=False)

    # ---- DRAM I/O ----
    xin = nc.dram_tensor("xin", [NB, CH, H + 2, PW], BF16, kind="ExternalInput")
    m8c = nc.dram_tensor("m8c", [msteps * NCHUNK, CC], BF16, kind="ExternalInput")
    w1t = nc.dram_tensor("w1t", [128, 3, HID], BF16, kind="ExternalInput")
    w2t = nc.dram_tensor("w2t", [HID, HID], BF16, kind="ExternalInput")
    w3t = nc.dram_tensor("w3t", [HID, CH], BF16, kind="ExternalInput")
    b1d = nc.dram_tensor("b1d", [HID, 1], F32, kind="ExternalInput")
    b2d = nc.dram_tensor("b2d", [HID, 1], F32, kind="ExternalInput")
    b3d = nc.dram_tensor("b3d", [128, 1], F32, kind="ExternalInput")
    seedd = nc.dram_tensor("seedd", [1], BF16, kind="ExternalInput")
    xout = nc.dram_tensor("xout", [NB, CH, H, W], F32, kind="ExternalOutput")

    with tile.TileContext(nc) as tc:
        with tc.tile_pool(name="state", bufs=1) as state, \
             tc.tile_pool(name="stkp", bufs=3) as stkp, \
             tc.tile_pool(name="h1p", bufs=3) as h1p, \
             tc.tile_pool(name="h2p", bufs=3) as h2p, \
             tc.tile_pool(name="ump", bufs=2) as ump, \
             tc.tile_pool(name="xnp", bufs=2) as xnp, \
             tc.tile_pool(name="mtp", bufs=2) as mtp, \
             tc.tile_pool(name="atp", bufs=2) as atp, \
             tc.tile_pool(name="mpp", bufs=1) as mpp, \
             tc.tile_pool(name="avp", bufs=2) as avp, \
             tc.tile_pool(name="dram", bufs=2, space="DRAM") as dramp, \
             tc.tile_pool(name="ps1p", bufs=2, space="PSUM") as ps1p, \
             tc.tile_pool(name="ps2p", bufs=2, space="PSUM") as ps2p, \
             tc.tile_pool(name="ps3p", bufs=2, space="PSUM") as ps3p:

            # ---- persistent state ----
            xck = [state.tile([128, PR, PW], BF16, tag=f"xck{k}", name=f"xck{k}")
                   for k in range(NCHUNK)]
            w1s = state.tile([128, 3, HID], BF16, tag="w1s")
            w2s = state.tile([HID, HID], BF16, tag="w2s")
            w3s = state.tile([HID, CH], BF16, tag="w3s")
            b1s = state.tile([HID, 1], F32, tag="b1s")
            b2s = state.tile([HID, 1], F32, tag="b2s")
            b3s = state.tile([128, 1], F32, tag="b3s")

            nc.sync.dma_start(out=w1s, in_=w1t[:])
            nc.sync.dma_start(out=w2s, in_=w2t[:])
            nc.sync.dma_start(out=w3s, in_=w3t[:])
            nc.sync.dma_start(out=b1s, in_=b1d[:])
            nc.sync.dma_start(out=b2s, in_=b2d[:])
            nc.sync.dma_start(out=b3s, in_=b3d[:])

            # load x: memset (zeros gap partitions) then per-image interior DMA
            for k in range(NCHUNK):
                nc.vector.memset(xck[k], 0.0)
                for b in range(NB):
                    nc.sync.dma_start(
                        out=xck[k][32 * b:32 * b + CH, :, :],
                        in_=xin[b, :, k * RCH:k * RCH + PR, :])

            def step_body(si):
                # ---- alive: gather alpha into compact [32, 18, PW] ----
                skip_alive = ("noalive2" in _dbg) or ("nomt" in _dbg)
                ac = mpp.tile([32, PR, PW], BF16, tag="ac")
                if not skip_alive:
                    for k in range(NCHUNK):
                        for b in range(NB):
                            nc.sync.dma_start(
                                out=ac[4 * k + b:4 * k + b + 1, :, :],
                                in_=xck[k][32 * b + 3:32 * b + 4, :, :])

                # maxpool 3x3 then threshold -> av [32, CC] bf16
                if skip_alive:
                    m1 = None
                else:
                    m1 = mpp.tile([32, PR, W], BF16, tag="m1")
                if not skip_alive:
                    nc.vector.tensor_max(m1, ac[:, :, 0:W], ac[:, :, 2:W + 2])
                    nc.vector.tensor_max(m1, m1, ac[:, :, 1:W + 1])
                    my = mpp.tile([32, RCH, W], BF16, tag="my")
                    nc.vector.tensor_max(my, m1[:, 0:RCH], m1[:, 2:RCH + 2])
                    nc.vector.tensor_max(my, my, m1[:, 1:RCH + 1])
                    av = avp.tile([32, CC], BF16, tag="av")
                    nc.vector.tensor_single_scalar(
                        av, my.rearrange("p r w -> p (r w)"), 0.01, OP.is_gt)
                    avd = dramp.tile([32, CC], BF16, tag="avd")
                    nc.sync.dma_start(out=avd, in_=av)

                # ---- chunks ----
                for k in range(NCHUNK):
                    # mask tile for this chunk, replicated to 128 partitions
                    mt = mtp.tile([128, CC], BF16, tag="mt")
                    at = atp.tile([128, CC], BF16, tag="at")
                    if "nomt" in _dbg:
                        nc.vector.memset(mt, 1.0)
                        nc.vector.memset(at, 1.0)
                    else:
                        if isinstance(si, int):
                            ii = si * NCHUNK + k
                            msrc = m8c[ii:ii + 1, :]
                        else:
                            msrc = m8c[bass.ds(si * NCHUNK + k, 1), :]
                        nc.scalar.dma_start(out=mt,
                                            in_=msrc.to_broadcast([128, CC]))
                        for b in range(NB):
                            nc.scalar.dma_start(
                                out=at[32 * b:32 * b + CH, :],
                                in_=avd[4 * k + b:4 * k + b + 1, :]
                                    .to_broadcast([CH, CC]))

                    # ---- build y-stacks (two image-pairs) ----
                    stks = []
                    for j in range(2):
                        stk = stkp.tile([128, RCH, PW], BF16, tag="stk")
                        if "nostk" not in _dbg:
                            for i in range(2):      # pair member
                                bb = 32 * (2 * j + i)
                                for d in range(3):  # dy block
                                    nc.sync.dma_start(
                                        out=stk[64 * i + 16 * d:
                                                64 * i + 16 * d + CH, :, :],
                                        in_=xck[k][bb:bb + CH, d:d + RCH, :])
                        stks.append(stk)

                    um = ump.tile([128, CC], BF16, tag="um")
                    if "nomm" in _dbg:
                        nc.vector.memset(um, 0.0)
                    for n in range(NSUB if "nomm" not in _dbg else 0):
                        r0 = n * (RCH // NSUB)          # first row of sub-tile
                        nr = RCH // NSUB                # rows per sub (4)
                        ps3 = ps3p.tile([128, SUB], F32, tag="ps3")
                        for j in range(2):
                            stk = stks[j]
                            for half in range(2):       # 0: rows 0-47, 1: 64-111
                                hb = 64 * half
                                ps1 = ps1p.tile([128, SUB], F32, tag="ps1")
                                for t in range(3):
                                    nc.tensor.matmul(
                                        ps1,
                                        w1s[hb:hb + 48, t, :],
                                        stk[hb:hb + 48, r0:r0 + nr, t:t + W],
                                        start=(t == 0), stop=(t == 2))
                                h1 = h1p.tile([128, SUB], BF16, tag="h1")
                                nc.scalar.activation(h1, ps1, AF.Relu, bias=b1s)
                                ps2 = ps2p.tile([128, SUB], F32, tag="ps2")
                                nc.tensor.matmul(ps2, w2s, h1,
                                                 start=True, stop=True)
                                h2 = h2p.tile([128, SUB], BF16, tag="h2")
                                nc.vector.tensor_scalar(
                                    h2, ps2, b2s, 0.0, OP.add, OP.max)
                                cp = 64 * j + 32 * half
                                nc.tensor.matmul(
                                    ps3[cp:cp + CH, :], w3s, h2,
                                    start=True, stop=True,
                                    tile_position=(0, cp))
                        # um = (ps3 + b3) * mask
                        nc.vector.scalar_tensor_tensor(
                            um[0:112, n * SUB:(n + 1) * SUB],
                            ps3[0:112, :], b3s[0:112, :],
                            mt[0:112, n * SUB:(n + 1) * SUB],
                            op0=OP.add, op1=OP.mult)

                    # ---- epilogue for this chunk ----
                    if "noepi" in _dbg:
                        continue
                    xi = xck[k][:, 1:RCH + 1, 1:W + 1]
                    xn = xnp.tile([128, CC], F32, tag="xn")
                    xn3 = xn.rearrange("p (r w) -> p r w", r=RCH)
                    um3 = um.rearrange("p (r w) -> p r w", r=RCH)
                    at3 = at.rearrange("p (r w) -> p r w", r=RCH)
                    if "noum" in _dbg:
                        nc.vector.tensor_copy(xn3, xi)
                    else:
                        nc.vector.tensor_add(xn3, xi, um3)
                    if "notanh" not in _dbg:
                        nc.scalar.activation(xn, xn, AF.Tanh)
                    if "noalive" in _dbg:
                        nc.vector.tensor_copy(xi, xn3)
                    else:
                        nc.vector.tensor_tensor(xi, xn3, at3, OP.mult)
                    if k == 2:      # seed pixel: image row 32 = local row 1
                        for b in range(NB):
                            nc.sync.dma_start(
                                out=xck[2][32 * b + 3:32 * b + CH, 1, 33:34],
                                in_=seedd[:].to_broadcast([CH - 3, 1]))

                # ---- step end: refresh inter-chunk halo rows ----
                for k in range(NCHUNK - 1):
                    # xck[k+1] top halo (image row 16(k+1)-1 = xck[k] local 16)
                    nc.sync.dma_start(out=xck[k + 1][:, 0, :],
                                      in_=xck[k][:, RCH, :])
                    # xck[k] bottom halo (image row 16(k+1)) = xck[k+1] local 1
                    nc.sync.dma_start(out=xck[k][:, RCH + 1, :],
                                      in_=xck[k + 1][:, 1, :])

            if loop_n is not None:
                with tc.For_i(0, loop_n, 1) as si:
                    step_body(si)
            else:
                for si in range(steps):
                    step_body(si)

            # ---- write out (strip padding; cast bf16->f32 via SWDGE) ----
            for k in range(NCHUNK):
                for b in range(NB):
                    nc.gpsimd.dma_start(
                        out=xout[b, :, k * RCH:(k + 1) * RCH, :],
                        in_=xck[k][32 * b:32 * b + CH, 1:RCH + 1, 1:W + 1])

    nc.compile()
    return nc


def _prep_weights(w1, b1, w2, b2, w3, b3):
    import ml_dtypes
    bf = ml_dtypes.bfloat16
    w1 = np.asarray(w1, np.float32)
    W1x, W1gx, W1gy, W1lap = (w1[:, 0:16], w1[:, 16:32],
                              w1[:, 32:48], w1[:, 48:64])
    # A[dy, dx, m, c] = tap weight matrices
    A = (KX[:, :, None, None] * W1gx[None, None] +
         KY[:, :, None, None] * W1gy[None, None] +
         KL[:, :, None, None] * W1lap[None, None])
    A = A.astype(np.float32).copy()
    A[1, 1] += W1x
    # w1t[dy*16+c, t, m] = A[dy, t, m, c]
    w1t = np.zeros((128, 3, HID), np.float32)
    blk = A.transpose(0, 3, 1, 2).reshape(48, 3, HID)
    w1t[0:48] = blk
    w1t[64:112] = blk
    w2t = np.ascontiguousarray(np.asarray(w2, np.float32).T)
    w3t = np.ascontiguousarray((0.05 * np.asarray(w3, np.float32)).T)
    b1v = np.asarray(b1, np.float32).reshape(HID, 1)
    b2v = np.asarray(b2, np.float32).reshape(HID, 1)
    b3v = np.zeros((128, 1), np.float32)
    for b in range(NB):
        b3v[32 * b:32 * b + CH, 0] = 0.05 * np.asarray(b3, np.float32)
    return (w1t.astype(bf), w2t.astype(bf), w3t.astype(bf), b1v, b2v, b3v)


LAST_RESULT = None


def kernel(x, rand, w1, b1, w2, b2, w3, b3, steps=STEPS, trace=False,
           tmpdir=None):
    global LAST_RESULT
    import ml_dtypes
    from concourse.bass_utils import run_bass_kernel_spmd

    import os
    key = (steps, os.environ.get("NCA_DBG", ""))
    if key not in _BUILD_CACHE:
        _BUILD_CACHE[key] = _build(steps)
    nc = _BUILD_CACHE[key]

    x = np.asarray(x, np.float32)
    rand = np.asarray(rand, np.float32)
    w1t, w2t, w3t, b1v, b2v, b3v = _prep_weights(w1, b1, w2, b2, w3, b3)
    m8c = (rand < 0.5).astype(np.float32).reshape(STEPS * NCHUNK, CC)
    if isinstance(steps, tuple):
        reps = -(-steps[1] * NCHUNK // m8c.shape[0])
        m8c = np.tile(m8c, (reps, 1))[:steps[1] * NCHUNK]
    m8c = m8c.astype(ml_dtypes.bfloat16)

    in_maps = []
    for c in range(NCORES):
        shard = x[c * NB:(c + 1) * NB]                 # [4, 16, H, W]
        xp = np.zeros((NB, CH, H + 2, PW), np.float32)
        xp[:, :, 1:H + 1, 1:W + 1] = shard
        xp = xp.astype(ml_dtypes.bfloat16)
        in_maps.append({
            "xin": xp, "m8c": m8c, "w1t": w1t, "w2t": w2t, "w3t": w3t,
            "b1d": b1v, "b2d": b2v, "b3d": b3v,
            "seedd": np.array([np.tanh(1.0)], ml_dtypes.bfloat16),
        })

    res = run_bass_kernel_spmd(nc, in_maps, core_ids=list(range(NCORES)),
                               trace=trace, tmpdir=tmpdir)
    LAST_RESULT = res
    out = np.concatenate([r["xout"] for r in res.results], axis=0)
    return out.astype(np.float32)

